# revision 1
# baseline (speedup 1.0000x reference)
"""Causal GQA self-attention (B=2,T=2048,C=4096, 32 q-heads, 8 kv-groups, hs=128)
sharded tensor-parallel across 8 TRN2 NeuronCores: one kv-group (4 q heads) per core.

Per-core pipeline (all activations feature-major, i.e. transposed):
  1. qkvT = Wqkv_g^T @ x^T           (bf16 matmul, fp32 psum)
     RoPE applied to q/k in fp32 during psum->sbuf epilogue, cast bf16
  2. v transposed to token-major via PE transpose
  3. per (batch, head): S^T = k'T^T-slices @ q'T  (scores transposed:
     kv-pos on partitions, q-pos on free dim), exp via ACT (scale folded),
     causal mask by multiply on diagonal tiles, O^T accumulated via
     v_tok^T @ P^T, denominator via ones^T @ P^T
  4. normalization: 1/denom via ACT exp(-ln(d)), broadcast across
     partitions via PE outer-product, multiply into O^T
  5. partial out = y^T-slices^T @ Wproj_g   -> [4096 tok, 4096] bf16
Host sums the 8 partial outputs in fp32.
"""
import math

import numpy as np
import ml_dtypes

import concourse.bass as bass
import concourse.mybir as mybir
import concourse.tile as tile
from concourse import bacc
from concourse.bass_utils import run_bass_kernel_spmd

BF16 = mybir.dt.bfloat16
F32 = mybir.dt.float32
AF = mybir.ActivationFunctionType

N_CORES = 8
B, T, C = 2, 2048, 4096
HS = 128
QPK = 4              # q heads per kv group
GCOLS = (QPK + 2) * HS   # 768 qkv columns per group
TOK = B * T          # 4096
SCALE = float(1.0 / np.sqrt(np.float32(HS)))

_NC_CACHE = None


def build_nc():
    nc = bacc.Bacc("TRN2", target_bir_lowering=False, debug=False,
                   num_devices=N_CORES)
    xT = nc.dram_tensor("xt", [C, TOK], BF16, kind="ExternalInput").ap()
    wqkv = nc.dram_tensor("wqkv", [C, GCOLS], BF16, kind="ExternalInput").ap()
    wproj = nc.dram_tensor("wproj", [QPK * HS, C], BF16, kind="ExternalInput").ap()
    cosf = nc.dram_tensor("cosf", [128, T], F32, kind="ExternalInput").ap()
    sinb = nc.dram_tensor("sinb", [128, T], F32, kind="ExternalInput").ap()
    masks = nc.dram_tensor("masks", [128, 4, 512], BF16, kind="ExternalInput").ap()
    onesc = nc.dram_tensor("onesc", [128, 1], BF16, kind="ExternalInput").ap()
    onesr = nc.dram_tensor("onesr", [1, 128], F32, kind="ExternalInput").ap()
    ident = nc.dram_tensor("ident", [128, 128], BF16, kind="ExternalInput").ap()
    out = nc.dram_tensor("out", [TOK, C], BF16, kind="ExternalOutput").ap()

    xT_r = xT.rearrange("(ko p) t -> p ko t", p=128)        # [128, 32, TOK]
    wqkv_r = wqkv.rearrange("(ko p) m -> p ko m", p=128)    # [128, 32, 768]
    wproj_r = wproj.rearrange("(h p) c -> p h c", p=128)    # [128, 4, C]

    with tile.TileContext(nc) as tc:
        with tc.tile_pool(name="mid", bufs=1) as mid:
            qT = mid.tile([128, QPK, TOK], BF16)   # rope'd q, feature-major
            kT = mid.tile([128, TOK], BF16)
            vT = mid.tile([128, TOK], BF16)

            # ---------------- stage 1: qkv + rope ----------------
            with tc.tile_pool(name="s1c", bufs=1) as s1c, \
                 tc.tile_pool(name="s1x", bufs=4) as s1x, \
                 tc.tile_pool(name="s1r", bufs=3) as s1r, \
                 tc.tile_pool(name="ps1", bufs=4, space="PSUM") as ps1:
                wq_sb = s1c.tile([128, 32, GCOLS], BF16)
                for kk in range(32):
                    nc.sync.dma_start(wq_sb[:, kk, :], wqkv_r[:, kk, :])
                cos_sb = s1c.tile([128, T], F32)
                sin_sb = s1c.tile([128, T], F32)

                for tcch in range(TOK // 512):       # 8 chunks of 512 tokens
                    t0 = tcch * 512
                    tb = t0 % T                      # position within batch
                    xa = s1x.tile([128, 16, 512], BF16, tag="x")
                    for q4 in range(4):
                        nc.sync.dma_start(
                            xa[:, q4 * 4:(q4 + 1) * 4, :],
                            xT_r[:, q4 * 4:(q4 + 1) * 4, t0:t0 + 512])
                    xb = s1x.tile([128, 16, 512], BF16, tag="x")
                    for q4 in range(4):
                        nc.sync.dma_start(
                            xb[:, q4 * 4:(q4 + 1) * 4, :],
                            xT_r[:, 16 + q4 * 4:16 + (q4 + 1) * 4, t0:t0 + 512])
                    if tcch == 0:
                        nc.sync.dma_start(cos_sb[:], cosf[:])
                        nc.sync.dma_start(sin_sb[:], sinb[:])
                    for m in range(6):
                        ps = ps1.tile([128, 512], F32, tag="ps")
                        for kh, xt_t in ((0, xa), (1, xb)):
                            for ki in range(16):
                                kk = kh * 16 + ki
                                nc.tensor.matmul(
                                    ps[:], wq_sb[:, kk, m * 128:(m + 1) * 128],
                                    xt_t[:, ki, :],
                                    start=(kk == 0), stop=(kk == 31))
                        if m == 5:                   # v: plain copy+cast
                            nc.vector.tensor_copy(vT[:, t0:t0 + 512], ps[:])
                        else:                        # q heads 0-3 / k: rope
                            t1 = s1r.tile([128, 512], F32, tag="t1")
                            nc.vector.tensor_mul(t1[:], ps[:],
                                                 cos_sb[:, tb:tb + 512])
                            u = s1r.tile([128, 512], F32, tag="u")
                            nc.vector.tensor_mul(u[0:64, :], ps[64:128, :],
                                                 sin_sb[0:64, tb:tb + 512])
                            nc.vector.tensor_mul(u[64:128, :], ps[0:64, :],
                                                 sin_sb[64:128, tb:tb + 512])
                            if m < 4:
                                dst = qT[:, m, t0:t0 + 512]
                            else:
                                dst = kT[:, t0:t0 + 512]
                            nc.vector.tensor_add(dst, t1[:], u[:])

            # ---------------- stages 2+3: attention ----------------
            with tc.tile_pool(name="s3c", bufs=1) as s3c, \
                 tc.tile_pool(name="s3p", bufs=8) as s3p:
                ident_sb = s3c.tile([128, 128], BF16)
                nc.sync.dma_start(ident_sb[:], ident[:])
                onesc_sb = s3c.tile([128, 1], BF16)
                nc.sync.dma_start(onesc_sb[:], onesc[:])
                onesr_sb = s3c.tile([1, 128], F32)
                nc.sync.dma_start(onesr_sb[:], onesr[:])
                masks_sb = s3c.tile([128, 4, 512], BF16)
                nc.sync.dma_start(masks_sb[:], masks[:])
                v_tok = s3c.tile([128, 32, 128], BF16)
                yT = s3c.tile([128, QPK, TOK], BF16)
                wp_sb = s3c.tile([128, 4, C], BF16)
                for hh in range(4):
                    nc.sync.dma_start(wp_sb[:, hh, :], wproj_r[:, hh, :])

                with tc.tile_pool(name="psT", bufs=4, space="PSUM") as psT:
                    # v -> token-major via PE transpose
                    for si in range(32):
                        pt_ps = psT.tile([128, 128], BF16, tag="tp")
                        nc.tensor.transpose(pt_ps[:],
                                            vT[:, si * 128:(si + 1) * 128],
                                            ident_sb[:])
                        nc.vector.tensor_copy(v_tok[:, si, :], pt_ps[:])

                psum_ctx = [tc.tile_pool(name="pss", bufs=3, space="PSUM"),
                            tc.tile_pool(name="pso", bufs=2, space="PSUM"),
                            tc.tile_pool(name="psd", bufs=2, space="PSUM"),
                            tc.tile_pool(name="psb", bufs=1, space="PSUM")]
                pss, pso, psd, psb = [c.__enter__() for c in psum_ctx]
                rtp_cm = tc.tile_pool(name="rtp", bufs=4)
                rtp = rtp_cm.__enter__()
                psum_ctx.append(rtp_cm)

                for b in range(B):
                    for tcq in range(4):              # 512-token q chunks
                        for h in range(QPK):
                            t0g = b * T + tcq * 512
                            n_s = (tcq + 1) * 4
                            ps_o = pso.tile([128, 512], F32, tag="o")
                            ps_d = psd.tile([1, 512], F32, tag="d")
                            for si in range(n_s):
                                s0g = b * T + si * 128
                                ps_s = pss.tile([128, 512], F32, tag="s")
                                nc.tensor.matmul(
                                    ps_s[:], kT[:, s0g:s0g + 128],
                                    qT[:, h, t0g:t0g + 512],
                                    start=True, stop=True)
                                pt = s3p.tile([128, 512], BF16, tag="pt")
                                nc.scalar.activation(pt[:], ps_s[:], AF.Exp,
                                                     scale=SCALE)
                                if si >= tcq * 4:
                                    j = si - tcq * 4
                                    nc.vector.tensor_mul(pt[:], pt[:],
                                                         masks_sb[:, j, :])
                                nc.tensor.matmul(
                                    ps_o[:], v_tok[:, b * 16 + si, :], pt[:],
                                    start=(si == 0), stop=(si == n_s - 1))
                                nc.tensor.matmul(
                                    ps_d[:], onesc_sb[:], pt[:],
                                    start=(si == 0), stop=(si == n_s - 1))
                            rden = rtp.tile([1, 512], F32, tag="rd")
                            nc.vector.reciprocal_approx_fast(rden[:], ps_d[:])
                            ps_bc = psb.tile([128, 512], F32, tag="bc")
                            nc.tensor.matmul(ps_bc[:], onesr_sb[:], rden[:],
                                             start=True, stop=True)
                            rb = rtp.tile([128, 512], F32, tag="rb")
                            nc.vector.tensor_copy(rb[:], ps_bc[:])
                            nc.vector.tensor_mul(yT[:, h, t0g:t0g + 512],
                                                 ps_o[:], rb[:])

                for c in reversed(psum_ctx):
                    c.__exit__(None, None, None)

                # ---------------- stage 4: proj ----------------
                with tc.tile_pool(name="s4o", bufs=4) as s4o, \
                     tc.tile_pool(name="ps4", bufs=4, space="PSUM") as ps4:
                    for ti in range(TOK // 128):
                        t0 = ti * 128
                        for cc in range(C // 512):
                            ps_p = ps4.tile([128, 512], F32, tag="p")
                            for h in range(QPK):
                                nc.tensor.matmul(
                                    ps_p[:], yT[:, h, t0:t0 + 128],
                                    wp_sb[:, h, cc * 512:(cc + 1) * 512],
                                    start=(h == 0), stop=(h == 3))
                            ob = s4o.tile([128, 512], BF16, tag="ob")
                            nc.vector.tensor_copy(ob[:], ps_p[:])
                            nc.sync.dma_start(
                                out[t0:t0 + 128, cc * 512:(cc + 1) * 512],
                                ob[:])
    nc.compile()
    return nc


def _prep_inputs(x, cos, sin, Wqkv, Wproj):
    bf = ml_dtypes.bfloat16
    xTn = np.ascontiguousarray(x.reshape(TOK, C).T).astype(bf)
    cosf = np.ascontiguousarray(cos.T.astype(np.float32))        # [128, T]
    sinT = cos_s = sin.T.astype(np.float32)
    sinb = np.concatenate([-sinT[0:64], sinT[64:128]], axis=0)
    sinb = np.ascontiguousarray(sinb, dtype=np.float32)
    p = np.arange(128)[:, None, None]
    j = np.arange(4)[None, :, None]
    f = np.arange(512)[None, None, :]
    masks = ((j * 128 + p) <= f).astype(bf)
    onesc = np.ones([128, 1], dtype=bf)
    onesr = np.ones([1, 128], dtype=np.float32)
    ident = np.eye(128, dtype=np.float32).astype(bf)
    in_maps = []
    for g in range(N_CORES):
        in_maps.append({
            "xt": xTn,
            "wqkv": np.ascontiguousarray(Wqkv[:, g * GCOLS:(g + 1) * GCOLS]).astype(bf),
            "wproj": np.ascontiguousarray(Wproj[g * 512:(g + 1) * 512, :]).astype(bf),
            "cosf": cosf, "sinb": sinb, "masks": masks,
            "onesc": onesc, "onesr": onesr, "ident": ident,
        })
    return in_maps


def kernel(x, cos, sin, Wqkv, Wproj, _trace=False):
    global _NC_CACHE
    x = np.asarray(x, dtype=np.float32)
    cos = np.asarray(cos, dtype=np.float32)
    sin = np.asarray(sin, dtype=np.float32)
    Wqkv = np.asarray(Wqkv, dtype=np.float32)
    Wproj = np.asarray(Wproj, dtype=np.float32)
    if _NC_CACHE is None:
        _NC_CACHE = build_nc()
    nc = _NC_CACHE
    in_maps = _prep_inputs(x, cos, sin, Wqkv, Wproj)
    res = run_bass_kernel_spmd(nc, in_maps, core_ids=list(range(N_CORES)),
                               trace=_trace)
    acc = np.zeros([TOK, C], dtype=np.float32)
    for r in res.results:
        acc += r["out"].astype(np.float32)
    if _trace:
        kernel._last_exec_ns = res.exec_time_ns
        kernel._last_trace = res.instructions_and_trace
    return acc.reshape(B, T, C)



# revision 6
# speedup vs baseline: 1.1801x; 1.1801x over previous
"""Causal GQA self-attention (B=2,T=2048,C=4096, 32 q-heads, 8 kv-groups, hs=128)
sharded tensor-parallel across 8 TRN2 NeuronCores: one kv-group (4 q heads) per core.

v2: single dense PE stream with software-pipelined fill scheduling.
  seg1: qkv+rope for b=0 chunks (dense), v-transposes interleaved
  seg2: b=0 attention, PE bubbles filled with b=1 qkv matmul groups
  seg3: b=1 attention, filled with b=0 output-projection groups
  seg4: remaining projection, dense
Attention per (b,tcq,h): scores computed in 4-slice "quads" ([128,4,512] psum,
one exp ACTIVATE over all 2048 cols), diagonal quads use shrinking-N matmuls
(triangular), tril mask on gpsimd, denominator broadcast via ones-matrix lhs,
normalize = reciprocal + multiply on DVE (no PE broadcast matmul).
Host sums the 8 partial outputs in fp32.
"""
import math

import numpy as np
import ml_dtypes

import concourse.bass as bass
import concourse.mybir as mybir
import concourse.tile as tile
from concourse import bacc
from concourse.bass_utils import run_bass_kernel_spmd

BF16 = mybir.dt.bfloat16
F32 = mybir.dt.float32
AF = mybir.ActivationFunctionType

N_CORES = 8
B, T, C = 2, 2048, 4096
HS = 128
QPK = 4                  # q heads per kv group
GCOLS = (QPK + 2) * HS   # 768 qkv columns per group
TOK = B * T              # 4096
NCH = TOK // 512         # 8 token chunks of 512
SCALE = float(1.0 / np.sqrt(np.float32(HS)))

_NC_CACHE = None


def build_nc():
    nc = bacc.Bacc("TRN2", target_bir_lowering=False, debug=False,
                   num_devices=N_CORES)
    # host-packed layouts (see _prep_inputs)
    xt = nc.dram_tensor("xt", [128, NCH, 32, 512], BF16, kind="ExternalInput").ap()
    wq = nc.dram_tensor("wq", [128, 6, 32, 128], BF16, kind="ExternalInput").ap()
    wp = nc.dram_tensor("wp", [128, QPK, C], BF16, kind="ExternalInput").ap()
    # cos/sin slices per in-batch 512-chunk, bf16, sin pre-negated on first half
    cs = nc.dram_tensor("cs", [128, 4, 2, 512], BF16, kind="ExternalInput").ap()
    tri = nc.dram_tensor("tri", [128, 128], BF16, kind="ExternalInput").ap()
    ones = nc.dram_tensor("ones", [128, 128], BF16, kind="ExternalInput").ap()
    ident = nc.dram_tensor("ident", [128, 128], BF16, kind="ExternalInput").ap()
    out = nc.dram_tensor("out", [TOK, C], BF16, kind="ExternalOutput").ap()

    with tile.TileContext(nc) as tc:
        mid_cm = tc.tile_pool(name="mid", bufs=1)
        mid = mid_cm.__enter__()
        s2_cm = tc.tile_pool(name="s2", bufs=1)
        s2 = s2_cm.__enter__()
        pf_cm = tc.tile_pool(name="pf", bufs=2, space="PSUM")
        pf = pf_cm.__enter__()
        pscr_cm = tc.tile_pool(name="pscr", bufs=1, space="PSUM")
        pscr = pscr_cm.__enter__()
        po_cm = tc.tile_pool(name="po", bufs=1, space="PSUM")
        po = po_cm.__enter__()
        pd_cm = tc.tile_pool(name="pd", bufs=1, space="PSUM")
        pd = pd_cm.__enter__()
        s1_cm = tc.tile_pool(name="s1", bufs=1)
        s1 = s1_cm.__enter__()

        # ---- persistent sbuf ----
        qT = mid.tile([128, QPK, TOK], BF16)     # rope'd q, feature-major
        kT = mid.tile([128, TOK], BF16)          # rope'd k, feature-major
        ident_sb = mid.tile([128, 128], BF16)
        tri_sb = mid.tile([128, 128], BF16)
        ones_sb = mid.tile([128, 128], BF16)
        v_tok = s2.tile([128, 32, 128], BF16)    # v token-major per 128-slice
        yT = s2.tile([128, QPK, TOK], BF16)      # attention out, feature-major
        wq_sb = s1.tile([128, 6, 32, 128], BF16)

        nc.sync.dma_start(ident_sb[:], ident[:])
        nc.sync.dma_start(tri_sb[:], tri[:])
        nc.sync.dma_start(ones_sb[:], ones[:])
        # prewarm the exp table on ACT while DMAs stream
        warm = s2.tile([128, 128], BF16, tag="warm")
        nc.scalar.activation(warm[:], tri_sb[:], AF.Exp, scale=1.0)

        state = {"x": {}, "cs": {}, "vtmp": {}, "wp": None}

        def dma_chunk(c):
            """Issue DMAs for chunk c's x (4 quarter tiles) + cos/sin."""
            qs = []
            for qi in range(4):
                xq = s1.tile([128, 8, 512], BF16, tag="x", bufs=6,
                             name=f"x{c}_{qi}")
                for g in range(2):
                    nc.sync.dma_start(xq[:, g * 4:(g + 1) * 4, :],
                                      xt[:, c, qi * 8 + g * 4:qi * 8 + (g + 1) * 4, :])
                qs.append(xq)
            cst = s1.tile([128, 2, 512], BF16, tag="cs", bufs=2, name=f"cs{c}")
            nc.sync.dma_start(cst[:], cs[:, c % 4, :, :])
            state["x"][c] = qs
            state["cs"][c] = cst

        # stage-1 weight m=0 first, then first x chunk, then the rest
        nc.sync.dma_start(wq_sb[:, 0, 0:16, :], wq[:, 0, 0:16, :])
        nc.sync.dma_start(wq_sb[:, 0, 16:32, :], wq[:, 0, 16:32, :])
        dma_chunk(0)
        for m in range(1, 6):
            for half in range(2):
                nc.sync.dma_start(wq_sb[:, m, half * 16:(half + 1) * 16, :],
                                  wq[:, m, half * 16:(half + 1) * 16, :])
        dma_chunk(1)

        def emit_s1_mm(c, m, kk, ps):
            xq = state["x"][c][kk // 8]
            nc.tensor.matmul(ps[:], wq_sb[:, m, kk, :], xq[:, kk % 8, :],
                             start=(kk == 0), stop=(kk == 31))

        def emit_s1_epilogue(c, m, ps):
            t0 = c * 512
            cst = state["cs"][c]
            if m == 5:                       # v: stage for transpose
                vtmp = s1.tile([128, 512], BF16, tag="vtmp", bufs=2,
                               name=f"vt{c}")
                nc.vector.tensor_copy(vtmp[:], ps[:])
                state["vtmp"][c] = vtmp
            else:                            # q heads 0-3 / k: rope
                t1 = s1.tile([128, 512], F32, tag="t1", bufs=3, name="t1")
                nc.vector.tensor_mul(t1[:], ps[:], cst[:, 0, :])
                u = s1.tile([128, 512], F32, tag="u", bufs=3, name="u")
                nc.vector.tensor_mul(u[0:64, :], ps[64:128, :],
                                     cst[0:64, 1, :])
                nc.vector.tensor_mul(u[64:128, :], ps[0:64, :],
                                     cst[64:128, 1, :])
                if m < 4:
                    dst = qT[:, m, t0:t0 + 512]
                else:
                    dst = kT[:, t0:t0 + 512]
                nc.vector.tensor_add(dst, t1[:], u[:])

        def emit_transpose_unit(c):
            """Transpose chunk c's v into token-major v_tok, scratching in the
            4-bank scores-quad psum tile (bitcast to bf16)."""
            vtmp = state["vtmp"][c]
            psq = pscr.tile([128, 4, 512], F32, tag="s", name=f"tp{c}")
            pb = psq.bitcast(BF16)           # [128, 4, 1024]
            for s in range(4):
                dst = pb[:, s, 0:128]
                nc.tensor.transpose(dst, vtmp[:, s * 128:(s + 1) * 128],
                                    ident_sb[:])
                nc.vector.tensor_copy(v_tok[:, c * 4 + s, :], dst)

        # fill queue: list of (kind, cycles, emit_fn)
        fillq = []

        def push_s1_chunk(c):
            if c + 1 < NCH:
                fillq.append(("dma", 0, lambda cc=c + 1: dma_chunk(cc)))
            for m in range(6):
                holder = {}
                for gi, (k0, k1) in enumerate(((0, 12), (12, 24), (24, 32))):
                    def fn(cc=c, mm=m, kk0=k0, kk1=k1, gi=gi, h=holder):
                        if gi == 0:
                            h["ps"] = pf.tile([128, 512], F32, tag="f",
                                              name=f"s1p{cc}_{mm}")
                        for kk in range(kk0, kk1):
                            emit_s1_mm(cc, mm, kk, h["ps"])
                        if kk1 == 32:
                            emit_s1_epilogue(cc, mm, h["ps"])
                    fillq.append(("s1", (k1 - k0) * 512, fn))
            fillq.append(("s1", 4 * 430, lambda cc=c: emit_transpose_unit(cc)))

        def push_proj(b, tcq):
            for ti in range(b * 16 + tcq * 4, b * 16 + tcq * 4 + 4):
                for ccg in range(8):
                    def fn(t=ti, cg=ccg):
                        wp_sb = state["wp"]
                        ps_p = pf.tile([128, 512], F32, tag="f",
                                       name=f"pj{t}_{cg}")
                        for h in range(QPK):
                            nc.tensor.matmul(
                                ps_p[:], yT[:, h, t * 128:(t + 1) * 128],
                                wp_sb[:, h, cg * 512:(cg + 1) * 512],
                                start=(h == 0), stop=(h == 3))
                        ob = state["s3"].tile([128, 512], BF16, tag="ob",
                                              bufs=4, name=f"ob{t}_{cg}")
                        if (t * 8 + cg) % 2 == 0:
                            nc.vector.tensor_copy(ob[:], ps_p[:])
                        else:
                            nc.scalar.activation(ob[:], ps_p[:], AF.Copy)
                        nc.sync.dma_start(
                            out[t * 128:(t + 1) * 128,
                                cg * 512:(cg + 1) * 512], ob[:])
                    fillq.append(("pj", 4 * 512, fn))

        def emit_fill(budget, kinds):
            spent = 0
            while fillq and spent < budget and fillq[0][0] in kinds:
                kind, cyc, fn = fillq.pop(0)
                fn()
                spent += cyc
            return spent

        # ---------------- attention group ----------------
        def attention_group(b, tcq, h, fill_budget, kinds):
            t0g = b * T + tcq * 512
            n_s = (tcq + 1) * 4
            ps_o = po.tile([128, 512], F32, tag="o", name=f"o{b}{tcq}{h}")
            ps_d = pd.tile([128, 512], F32, tag="d", name=f"d{b}{tcq}{h}")
            for q in range(tcq + 1):
                diag = (q == tcq)
                ps_s = pscr.tile([128, 4, 512], F32, tag="s",
                                 name=f"s{b}{tcq}{h}{q}")
                offs = []
                for j in range(4):
                    si = q * 4 + j
                    off = 128 * j if diag else 0
                    offs.append(off)
                    s0g = b * T + si * 128
                    nc.tensor.matmul(
                        ps_s[:, j, off:512], kT[:, s0g:s0g + 128],
                        qT[:, h, t0g + off:t0g + 512],
                        start=True, stop=True)
                pt = s2.tile([128, 4, 512], BF16, tag="pt", bufs=2,
                             name=f"pt{b}{tcq}{h}{q}")
                nc.scalar.activation(pt[:], ps_s[:], AF.Exp, scale=SCALE)
                emit_fill(fill_budget, kinds)
                if diag:
                    for j in range(4):
                        o = 128 * j
                        nc.gpsimd.tensor_mul(pt[:, j, o:o + 128],
                                             pt[:, j, o:o + 128], tri_sb[:])
                for j in range(4):
                    si = q * 4 + j
                    off = offs[j]
                    nc.tensor.matmul(
                        ps_o[:, off:512], v_tok[:, b * 16 + si, :],
                        pt[:, j, off:512],
                        start=(si == 0), stop=(si == n_s - 1))
                    nc.tensor.matmul(
                        ps_d[:, off:512], ones_sb[:], pt[:, j, off:512],
                        start=(si == 0), stop=(si == n_s - 1))
            rden = s2.tile([128, 512], F32, tag="rd", bufs=1, name="rden")
            nc.vector.reciprocal_approx_fast(rden[:], ps_d[:])
            nc.vector.tensor_mul(yT[:, h, t0g:t0g + 512], ps_o[:], rden[:])

        # ================= emission =================
        # seg1: chunks 0-3 dense (b=0 qkv)
        for c in range(4):
            for m in range(6):
                if m == 0 and c + 2 < 4:
                    dma_chunk(c + 2)
                ps = pf.tile([128, 512], F32, tag="f", name=f"c{c}m{m}")
                for kk in range(32):
                    emit_s1_mm(c, m, kk, ps)
                emit_s1_epilogue(c, m, ps)
                if m == 1 and c >= 1:
                    emit_transpose_unit(c - 1)
        emit_transpose_unit(3)

        # queue b=1 qkv as fill for seg2; prefetch chunk 4 now
        dma_chunk(4)
        for c in range(4, 8):
            push_s1_chunk(c)

        s1_cycles = sum(cyc for _, cyc, _ in fillq)
        n_quads = QPK * sum(tcq + 1 for tcq in range(4))   # 40
        budget0 = s1_cycles // n_quads + 1

        # seg2: b=0 attention + b=1 qkv fills
        for tcq in range(4):
            for h in range(QPK):
                attention_group(0, tcq, h, budget0, ("dma", "s1"))
            push_proj(0, tcq)

        # make sure all b=1 qkv work is emitted before b=1 attention
        while fillq and fillq[0][0] in ("dma", "s1"):
            _, _, fn = fillq.pop(0)
            fn()

        # close stage-1 pool, open proj pool; wproj arrives during b=1 attn
        s1_cm.__exit__(None, None, None)
        s3_cm = tc.tile_pool(name="s3", bufs=1)
        s3 = s3_cm.__enter__()
        state["s3"] = s3
        wp_sb = s3.tile([128, QPK, C], BF16)
        state["wp"] = wp_sb
        for h in range(QPK):
            nc.sync.dma_start(wp_sb[:, h, :], wp[:, h, :])

        pj_cycles = sum(cyc for _, cyc, _ in fillq)
        budget1 = pj_cycles // n_quads + 1

        # seg3: b=1 attention + b=0 proj fills
        for tcq in range(4):
            for h in range(QPK):
                attention_group(1, tcq, h, budget1, ("dma", "s1", "pj"))
            push_proj(1, tcq)

        # seg4: drain remaining proj
        while fillq:
            _, _, fn = fillq.pop(0)
            fn()

        for cm in (s3_cm, pd_cm, po_cm, pscr_cm, pf_cm, s2_cm, mid_cm):
            cm.__exit__(None, None, None)
    nc.compile()
    return nc


def _prep_inputs(x, cos, sin, Wqkv, Wproj):
    bf = ml_dtypes.bfloat16
    # x: [B,T,C] -> xT [C, TOK] -> [128p, chunk, 32ko, 512]
    xTn = x.reshape(TOK, C).T.astype(bf)                  # [C, TOK]
    xpack = np.ascontiguousarray(
        xTn.reshape(32, 128, NCH, 512).transpose(1, 2, 0, 3))
    # cos/sin: [T, 128] -> feature-major slices [128, 4tcq, 2, 512]
    cosT = cos.T.astype(np.float32)                       # [128, T]
    sinT = sin.T.astype(np.float32)
    sinb = np.concatenate([-sinT[0:64], sinT[64:128]], axis=0)
    cspack = np.empty((128, 4, 2, 512), dtype=np.float32)
    for tc in range(4):
        cspack[:, tc, 0, :] = cosT[:, tc * 512:(tc + 1) * 512]
        cspack[:, tc, 1, :] = sinb[:, tc * 512:(tc + 1) * 512]
    cspack = cspack.astype(bf)
    p = np.arange(128)[:, None]
    f = np.arange(128)[None, :]
    tri = (p <= f).astype(bf)                             # tril mask (kv<=q)
    ones = np.ones([128, 128], dtype=bf)
    ident = np.eye(128, dtype=np.float32).astype(bf)
    in_maps = []
    for g in range(N_CORES):
        Wg = np.ascontiguousarray(Wqkv[:, g * GCOLS:(g + 1) * GCOLS])
        # [C, 768] -> [128p, 6m, 32ko, 128]
        wqp = np.ascontiguousarray(
            Wg.reshape(32, 128, 6, 128).transpose(1, 2, 0, 3).astype(bf))
        Wpg = Wproj[g * 512:(g + 1) * 512, :]             # [512, C]
        wpp = np.ascontiguousarray(
            Wpg.reshape(QPK, 128, C).transpose(1, 0, 2).astype(bf))
        in_maps.append({
            "xt": xpack, "wq": wqp, "wp": wpp, "cs": cspack,
            "tri": tri, "ones": ones, "ident": ident,
        })
    return in_maps


def kernel(x, cos, sin, Wqkv, Wproj, _trace=False):
    global _NC_CACHE
    x = np.asarray(x, dtype=np.float32)
    cos = np.asarray(cos, dtype=np.float32)
    sin = np.asarray(sin, dtype=np.float32)
    Wqkv = np.asarray(Wqkv, dtype=np.float32)
    Wproj = np.asarray(Wproj, dtype=np.float32)
    if _NC_CACHE is None:
        _NC_CACHE = build_nc()
    nc = _NC_CACHE
    in_maps = _prep_inputs(x, cos, sin, Wqkv, Wproj)
    res = run_bass_kernel_spmd(nc, in_maps, core_ids=list(range(N_CORES)),
                               trace=_trace)
    acc = np.zeros([TOK, C], dtype=np.float32)
    for r in res.results:
        acc += r["out"].astype(np.float32)
    if _trace:
        kernel._last_exec_ns = res.exec_time_ns
        kernel._last_trace = res.instructions_and_trace
    return acc.reshape(B, T, C)


# revision 14
# speedup vs baseline: 1.1917x; 1.0099x over previous
"""Causal GQA self-attention (B=2,T=2048,C=4096, 32 q-heads, 8 kv-groups, hs=128)
sharded tensor-parallel across 8 TRN2 NeuronCores: one kv-group (4 q heads) per core.

v2: single dense PE stream with software-pipelined fill scheduling.
  seg1: qkv+rope for b=0 chunks (dense), v-transposes interleaved
  seg2: b=0 attention, PE bubbles filled with b=1 qkv matmul groups
  seg3: b=1 attention, filled with b=0 output-projection groups
  seg4: remaining projection, dense
Attention per (b,tcq,h): scores computed in 4-slice "quads" ([128,4,512] psum,
one exp ACTIVATE over all 2048 cols), diagonal quads use shrinking-N matmuls
(triangular), tril mask on gpsimd, denominator broadcast via ones-matrix lhs,
normalize = reciprocal + multiply on DVE (no PE broadcast matmul).
Host sums the 8 partial outputs in fp32.
"""
import math

import numpy as np
import ml_dtypes

import concourse.bass as bass
import concourse.mybir as mybir
import concourse.tile as tile
from concourse import bacc
from concourse.bass_utils import run_bass_kernel_spmd

BF16 = mybir.dt.bfloat16
F32 = mybir.dt.float32
AF = mybir.ActivationFunctionType

N_CORES = 8
B, T, C = 2, 2048, 4096
HS = 128
QPK = 4                  # q heads per kv group
GCOLS = (QPK + 2) * HS   # 768 qkv columns per group
TOK = B * T              # 4096
NCH = TOK // 512         # 8 token chunks of 512
SCALE = float(1.0 / np.sqrt(np.float32(HS)))

_NC_CACHE = None


def build_nc():
    nc = bacc.Bacc("TRN2", target_bir_lowering=False, debug=False,
                   num_devices=N_CORES)
    # host-packed layouts (see _prep_inputs)
    xt = nc.dram_tensor("xt", [128, NCH, 32, 512], BF16, kind="ExternalInput").ap()
    wq = nc.dram_tensor("wq", [128, 6, 32, 128], BF16, kind="ExternalInput").ap()
    wp = nc.dram_tensor("wp", [128, QPK, C], BF16, kind="ExternalInput").ap()
    # cos/sin slices per in-batch 512-chunk, bf16, sin pre-negated on first half
    cs = nc.dram_tensor("cs", [128, 4, 2, 512], BF16, kind="ExternalInput").ap()
    tri = nc.dram_tensor("tri", [128, 128], BF16, kind="ExternalInput").ap()
    ones = nc.dram_tensor("ones", [128, 128], BF16, kind="ExternalInput").ap()
    ident = nc.dram_tensor("ident", [128, 128], BF16, kind="ExternalInput").ap()
    out = nc.dram_tensor("out", [TOK, C], BF16, kind="ExternalOutput").ap()

    with tile.TileContext(nc) as tc:
        mid_cm = tc.tile_pool(name="mid", bufs=1)
        mid = mid_cm.__enter__()
        s2_cm = tc.tile_pool(name="s2", bufs=1)
        s2 = s2_cm.__enter__()
        pf_cm = tc.tile_pool(name="pf", bufs=2, space="PSUM")
        pf = pf_cm.__enter__()
        pscr_cm = tc.tile_pool(name="pscr", bufs=1, space="PSUM")
        pscr = pscr_cm.__enter__()
        po_cm = tc.tile_pool(name="po", bufs=1, space="PSUM")
        po = po_cm.__enter__()
        pd_cm = tc.tile_pool(name="pd", bufs=1, space="PSUM")
        pd = pd_cm.__enter__()
        s1_cm = tc.tile_pool(name="s1", bufs=1)
        s1 = s1_cm.__enter__()

        # ---- persistent sbuf ----
        qT = mid.tile([128, QPK, TOK], BF16)     # rope'd q, feature-major
        kT = mid.tile([128, TOK], BF16)          # rope'd k, feature-major
        ident_sb = mid.tile([128, 128], BF16)
        tri_sb = mid.tile([128, 128], BF16)
        ones_sb = mid.tile([128, 128], BF16)
        v_tok = s2.tile([128, 32, 128], BF16)    # v token-major per 128-slice
        yT = s2.tile([128, QPK, TOK], BF16)      # attention out, feature-major
        wq_sb = s1.tile([128, 6, 32, 128], BF16)

        nc.sync.dma_start(ident_sb[:], ident[:])
        nc.sync.dma_start(tri_sb[:], tri[:])
        nc.sync.dma_start(ones_sb[:], ones[:])
        # prewarm the exp table on ACT while DMAs stream
        warm = s2.tile([128, 128], BF16, tag="warm")
        nc.scalar.activation(warm[:], tri_sb[:], AF.Exp, scale=1.0)

        state = {"x": {}, "cs": {}, "vtmp": {}, "wp": None}

        def dma_chunk(c):
            """Issue DMAs for chunk c's x (8 eighth tiles) + cos/sin."""
            qs = []
            for qi in range(8):
                xq = s1.tile([128, 4, 512], BF16, tag="x", bufs=13,
                             name=f"x{c}_{qi}")
                nc.sync.dma_start(xq[:], xt[:, c, qi * 4:(qi + 1) * 4, :])
                qs.append(xq)
            cst = s1.tile([128, 2, 512], BF16, tag="cs", bufs=2, name=f"cs{c}")
            nc.sync.dma_start(cst[:], cs[:, c % 4, :, :])
            state["x"][c] = qs
            state["cs"][c] = cst

        # stage-1 weights m=0/1 first (finely split), then first x chunk
        for m in (0, 1):
            for qr in range(4):
                nc.sync.dma_start(wq_sb[:, m, qr * 8:(qr + 1) * 8, :],
                                  wq[:, m, qr * 8:(qr + 1) * 8, :])
        dma_chunk(0)
        for m in range(2, 6):
            for half in range(2):
                nc.sync.dma_start(wq_sb[:, m, half * 16:(half + 1) * 16, :],
                                  wq[:, m, half * 16:(half + 1) * 16, :])
        dma_chunk(1)

        def emit_s1_mm(c, m, kk, ps):
            xq = state["x"][c][kk // 4]
            nc.tensor.matmul(ps[:], wq_sb[:, m, kk, :], xq[:, kk % 4, :],
                             start=(kk == 0), stop=(kk == 31))

        def emit_s1_epilogue(c, m, ps):
            t0 = c * 512
            cst = state["cs"][c]
            if m == 5:                       # v: stage for transpose
                vtmp = s1.tile([128, 512], BF16, tag="vtmp", bufs=2,
                               name=f"vt{c}")
                nc.vector.tensor_copy(vtmp[:], ps[:])
                state["vtmp"][c] = vtmp
            else:                            # q heads 0-3 / k: rope
                t1 = s1.tile([128, 512], F32, tag="t1", bufs=2, name="t1")
                nc.vector.tensor_mul(t1[:], ps[:], cst[:, 0, :])
                u = s1.tile([128, 512], F32, tag="u", bufs=2, name="u")
                nc.vector.tensor_mul(u[0:64, :], ps[64:128, :],
                                     cst[0:64, 1, :])
                nc.vector.tensor_mul(u[64:128, :], ps[0:64, :],
                                     cst[64:128, 1, :])
                if m < 4:
                    dst = qT[:, m, t0:t0 + 512]
                else:
                    dst = kT[:, t0:t0 + 512]
                nc.vector.tensor_add(dst, t1[:], u[:])

        def emit_transpose_unit(c):
            """Transpose chunk c's v into token-major v_tok, scratching in the
            4-bank scores-quad psum tile (bitcast to bf16)."""
            vtmp = state["vtmp"][c]
            psq = pscr.tile([128, 4, 512], F32, tag="s", name=f"tp{c}")
            pb = psq.bitcast(BF16)           # [128, 4, 1024]
            for s in range(4):
                dst = pb[:, s, 0:128]
                nc.tensor.transpose(dst, vtmp[:, s * 128:(s + 1) * 128],
                                    ident_sb[:])
                nc.vector.tensor_copy(v_tok[:, c * 4 + s, :], dst)

        # fill queue: list of (kind, cycles, emit_fn)
        fillq = []

        def push_s1_chunk(c):
            if c + 1 < NCH:
                fillq.append(("dma", 0, lambda cc=c + 1: dma_chunk(cc)))
            for m in range(6):
                holder = {}
                for gi, (k0, k1) in enumerate(((0, 12), (12, 24), (24, 32))):
                    def fn(cc=c, mm=m, kk0=k0, kk1=k1, gi=gi, h=holder):
                        if gi == 0:
                            h["ps"] = pf.tile([128, 512], F32, tag="f",
                                              name=f"s1p{cc}_{mm}")
                        for kk in range(kk0, kk1):
                            emit_s1_mm(cc, mm, kk, h["ps"])
                        if kk1 == 32:
                            emit_s1_epilogue(cc, mm, h["ps"])
                    fillq.append(("s1", (k1 - k0) * 512, fn))
            fillq.append(("s1", 4 * 430, lambda cc=c: emit_transpose_unit(cc)))

        def push_proj(b, tcq):
            for ti in range(b * 16 + tcq * 4, b * 16 + tcq * 4 + 4):
                for ccg in range(8):
                    def fn(t=ti, cg=ccg):
                        wp_sb = state["wp"]
                        ps_p = pf.tile([128, 512], F32, tag="f",
                                       name=f"pj{t}_{cg}")
                        for h in range(QPK):
                            nc.tensor.matmul(
                                ps_p[:], yT[:, h, t * 128:(t + 1) * 128],
                                wp_sb[:, h, cg * 512:(cg + 1) * 512],
                                start=(h == 0), stop=(h == 3))
                        ob = state["s3"].tile([128, 512], BF16, tag="ob",
                                              bufs=4, name=f"ob{t}_{cg}")
                        if (t * 8 + cg) % 2 == 0:
                            nc.vector.tensor_copy(ob[:], ps_p[:])
                        else:
                            nc.scalar.activation(ob[:], ps_p[:], AF.Copy)
                        nc.sync.dma_start(
                            out[t * 128:(t + 1) * 128,
                                cg * 512:(cg + 1) * 512], ob[:])
                    fillq.append(("pj", 4 * 512, fn))

        def emit_fill(budget, kinds):
            spent = 0
            while fillq and spent < budget and fillq[0][0] in kinds:
                kind, cyc, fn = fillq.pop(0)
                fn()
                spent += cyc
            return spent

        # ---------------- attention group ----------------
        def attention_group(b, tcq, h, fill_budget, kinds):
            t0g = b * T + tcq * 512
            n_s = (tcq + 1) * 4
            ps_o = po.tile([128, 512], F32, tag="o", name=f"o{b}{tcq}{h}")
            ps_d = pd.tile([128, 512], F32, tag="d", name=f"d{b}{tcq}{h}")
            for q in range(tcq + 1):
                diag = (q == tcq)
                ps_s = pscr.tile([128, 4, 512], F32, tag="s",
                                 name=f"s{b}{tcq}{h}{q}")
                offs = []
                for j in range(4):
                    si = q * 4 + j
                    off = 128 * j if diag else 0
                    offs.append(off)
                    s0g = b * T + si * 128
                    nc.tensor.matmul(
                        ps_s[:, j, off:512], kT[:, s0g:s0g + 128],
                        qT[:, h, t0g + off:t0g + 512],
                        start=True, stop=True)
                pt = s2.tile([128, 4, 512], BF16, tag="pt", bufs=2,
                             name=f"pt{b}{tcq}{h}{q}")
                nc.scalar.activation(pt[:], ps_s[:], AF.Exp, scale=SCALE)
                emit_fill(fill_budget, kinds)
                if diag:
                    for j in range(4):
                        o = 128 * j
                        nc.gpsimd.tensor_mul(pt[:, j, o:o + 128],
                                             pt[:, j, o:o + 128], tri_sb[:])
                for j in range(4):
                    si = q * 4 + j
                    off = offs[j]
                    nc.tensor.matmul(
                        ps_o[:, off:512], v_tok[:, b * 16 + si, :],
                        pt[:, j, off:512],
                        start=(si == 0), stop=(si == n_s - 1))
                    nc.tensor.matmul(
                        ps_d[:, off:512], ones_sb[:], pt[:, j, off:512],
                        start=(si == 0), stop=(si == n_s - 1))
            rden = s2.tile([128, 512], F32, tag="rd", bufs=1, name="rden")
            nc.vector.reciprocal_approx_fast(rden[:], ps_d[:])
            nc.vector.tensor_mul(yT[:, h, t0g:t0g + 512], ps_o[:], rden[:])

        # ================= emission =================
        # seg1: chunks 0-3 dense (b=0 qkv).  m-tiles processed in pairs with
        # split k-halves so late-arriving x quarters get 2x the DMA lead.
        for c in range(4):
            for ma, mb in ((0, 1), (2, 3), (4, 5)):
                if ma == 0 and c + 2 < 4:
                    dma_chunk(c + 2)
                psa = pf.tile([128, 512], F32, tag="f", name=f"c{c}m{ma}")
                psb = pf.tile([128, 512], F32, tag="f", name=f"c{c}m{mb}")
                for kk in range(16):
                    emit_s1_mm(c, ma, kk, psa)
                for kk in range(16):
                    emit_s1_mm(c, mb, kk, psb)
                for kk in range(16, 32):
                    emit_s1_mm(c, ma, kk, psa)
                emit_s1_epilogue(c, ma, psa)
                for kk in range(16, 32):
                    emit_s1_mm(c, mb, kk, psb)
                emit_s1_epilogue(c, mb, psb)
                if ma == 2 and c >= 1:
                    emit_transpose_unit(c - 1)
        emit_transpose_unit(3)

        # queue b=1 qkv as fill for seg2; prefetch chunk 4 now
        dma_chunk(4)
        for c in range(4, 8):
            push_s1_chunk(c)

        s1_cycles = sum(cyc for _, cyc, _ in fillq)
        n_quads = QPK * sum(tcq + 1 for tcq in range(4))   # 40
        budget0 = s1_cycles // n_quads + 1

        # seg2: b=0 attention + b=1 qkv fills
        for tcq in range(4):
            for h in range(QPK):
                attention_group(0, tcq, h, budget0, ("dma", "s1"))
            push_proj(0, tcq)

        # make sure all b=1 qkv work is emitted before b=1 attention
        while fillq and fillq[0][0] in ("dma", "s1"):
            _, _, fn = fillq.pop(0)
            fn()

        # close stage-1 pool, open proj pool; wproj arrives during b=1 attn
        s1_cm.__exit__(None, None, None)
        s3_cm = tc.tile_pool(name="s3", bufs=1)
        s3 = s3_cm.__enter__()
        state["s3"] = s3
        wp_sb = s3.tile([128, QPK, C], BF16)
        state["wp"] = wp_sb
        for h in range(QPK):
            for half in range(2):
                nc.sync.dma_start(wp_sb[:, h, half * 2048:(half + 1) * 2048],
                                  wp[:, h, half * 2048:(half + 1) * 2048])

        pj_cycles = sum(cyc for _, cyc, _ in fillq)
        budget1 = pj_cycles // n_quads + 1

        # seg3: b=1 attention + b=0 proj fills
        for tcq in range(4):
            for h in range(QPK):
                attention_group(1, tcq, h, budget1, ("dma", "s1", "pj"))
            push_proj(1, tcq)

        # seg4: drain remaining proj
        while fillq:
            _, _, fn = fillq.pop(0)
            fn()

        for cm in (s3_cm, pd_cm, po_cm, pscr_cm, pf_cm, s2_cm, mid_cm):
            cm.__exit__(None, None, None)
    nc.compile()
    return nc


def _prep_inputs(x, cos, sin, Wqkv, Wproj):
    bf = ml_dtypes.bfloat16
    # x: [B,T,C] -> xT [C, TOK] -> [128p, chunk, 32ko, 512]
    xTn = x.reshape(TOK, C).T.astype(bf)                  # [C, TOK]
    xpack = np.ascontiguousarray(
        xTn.reshape(32, 128, NCH, 512).transpose(1, 2, 0, 3))
    # cos/sin: [T, 128] -> feature-major slices [128, 4tcq, 2, 512]
    cosT = cos.T.astype(np.float32)                       # [128, T]
    sinT = sin.T.astype(np.float32)
    sinb = np.concatenate([-sinT[0:64], sinT[64:128]], axis=0)
    cspack = np.empty((128, 4, 2, 512), dtype=np.float32)
    for tc in range(4):
        cspack[:, tc, 0, :] = cosT[:, tc * 512:(tc + 1) * 512]
        cspack[:, tc, 1, :] = sinb[:, tc * 512:(tc + 1) * 512]
    cspack = cspack.astype(bf)
    p = np.arange(128)[:, None]
    f = np.arange(128)[None, :]
    tri = (p <= f).astype(bf)                             # tril mask (kv<=q)
    ones = np.ones([128, 128], dtype=bf)
    ident = np.eye(128, dtype=np.float32).astype(bf)
    in_maps = []
    for g in range(N_CORES):
        Wg = np.ascontiguousarray(Wqkv[:, g * GCOLS:(g + 1) * GCOLS])
        # [C, 768] -> [128p, 6m, 32ko, 128]
        wqp = np.ascontiguousarray(
            Wg.reshape(32, 128, 6, 128).transpose(1, 2, 0, 3).astype(bf))
        Wpg = Wproj[g * 512:(g + 1) * 512, :]             # [512, C]
        wpp = np.ascontiguousarray(
            Wpg.reshape(QPK, 128, C).transpose(1, 0, 2).astype(bf))
        in_maps.append({
            "xt": xpack, "wq": wqp, "wp": wpp, "cs": cspack,
            "tri": tri, "ones": ones, "ident": ident,
        })
    return in_maps


def kernel(x, cos, sin, Wqkv, Wproj, _trace=False):
    global _NC_CACHE
    x = np.asarray(x, dtype=np.float32)
    cos = np.asarray(cos, dtype=np.float32)
    sin = np.asarray(sin, dtype=np.float32)
    Wqkv = np.asarray(Wqkv, dtype=np.float32)
    Wproj = np.asarray(Wproj, dtype=np.float32)
    if _NC_CACHE is None:
        _NC_CACHE = build_nc()
    nc = _NC_CACHE
    in_maps = _prep_inputs(x, cos, sin, Wqkv, Wproj)
    res = run_bass_kernel_spmd(nc, in_maps, core_ids=list(range(N_CORES)),
                               trace=_trace)
    acc = np.zeros([TOK, C], dtype=np.float32)
    for r in res.results:
        acc += r["out"].astype(np.float32)
    if _trace:
        kernel._last_exec_ns = res.exec_time_ns
        kernel._last_trace = res.instructions_and_trace
    return acc.reshape(B, T, C)


# revision 17
# speedup vs baseline: 1.1922x; 1.0004x over previous
"""Causal GQA self-attention (B=2,T=2048,C=4096, 32 q-heads, 8 kv-groups, hs=128)
sharded tensor-parallel across 8 TRN2 NeuronCores: one kv-group (4 q heads) per core.

v2: single dense PE stream with software-pipelined fill scheduling.
  seg1: qkv+rope for b=0 chunks (dense), v-transposes interleaved
  seg2: b=0 attention, PE bubbles filled with b=1 qkv matmul groups
  seg3: b=1 attention, filled with b=0 output-projection groups
  seg4: remaining projection, dense
Attention per (b,tcq,h): scores computed in 4-slice "quads" ([128,4,512] psum,
one exp ACTIVATE over all 2048 cols), diagonal quads use shrinking-N matmuls
(triangular), tril mask on gpsimd, denominator broadcast via ones-matrix lhs,
normalize = reciprocal + multiply on DVE (no PE broadcast matmul).
Host sums the 8 partial outputs in fp32.
"""
import math

import numpy as np
import ml_dtypes

import concourse.bass as bass
import concourse.mybir as mybir
import concourse.tile as tile
from concourse import bacc
from concourse.bass_utils import run_bass_kernel_spmd

BF16 = mybir.dt.bfloat16
F32 = mybir.dt.float32
AF = mybir.ActivationFunctionType

N_CORES = 8
B, T, C = 2, 2048, 4096
HS = 128
QPK = 4                  # q heads per kv group
GCOLS = (QPK + 2) * HS   # 768 qkv columns per group
TOK = B * T              # 4096
NCH = TOK // 512         # 8 token chunks of 512
SCALE = float(1.0 / np.sqrt(np.float32(HS)))

_NC_CACHE = None


def build_nc():
    nc = bacc.Bacc("TRN2", target_bir_lowering=False, debug=False,
                   num_devices=N_CORES)
    # host-packed layouts (see _prep_inputs)
    xt = nc.dram_tensor("xt", [128, NCH, 32, 512], BF16, kind="ExternalInput").ap()
    wq = nc.dram_tensor("wq", [128, 6, 32, 128], BF16, kind="ExternalInput").ap()
    wp = nc.dram_tensor("wp", [128, QPK, C], BF16, kind="ExternalInput").ap()
    # cos/sin slices per in-batch 512-chunk, bf16, sin pre-negated on first half
    cs = nc.dram_tensor("cs", [128, 4, 2, 512], BF16, kind="ExternalInput").ap()
    tri = nc.dram_tensor("tri", [128, 128], BF16, kind="ExternalInput").ap()
    ones = nc.dram_tensor("ones", [128, 128], BF16, kind="ExternalInput").ap()
    ident = nc.dram_tensor("ident", [128, 128], BF16, kind="ExternalInput").ap()
    out = nc.dram_tensor("out", [TOK, C], BF16, kind="ExternalOutput").ap()

    with tile.TileContext(nc) as tc:
        mid_cm = tc.tile_pool(name="mid", bufs=1)
        mid = mid_cm.__enter__()
        s2_cm = tc.tile_pool(name="s2", bufs=1)
        s2 = s2_cm.__enter__()
        pf_cm = tc.tile_pool(name="pf", bufs=2, space="PSUM")
        pf = pf_cm.__enter__()
        pscr_cm = tc.tile_pool(name="pscr", bufs=1, space="PSUM")
        pscr = pscr_cm.__enter__()
        po_cm = tc.tile_pool(name="po", bufs=1, space="PSUM")
        po = po_cm.__enter__()
        pd_cm = tc.tile_pool(name="pd", bufs=1, space="PSUM")
        pd = pd_cm.__enter__()
        s1_cm = tc.tile_pool(name="s1", bufs=1)
        s1 = s1_cm.__enter__()

        # ---- persistent sbuf ----
        qT = mid.tile([128, QPK, TOK], BF16)     # rope'd q, feature-major
        kT = mid.tile([128, TOK], BF16)          # rope'd k, feature-major
        ident_sb = mid.tile([128, 128], BF16)
        tri_sb = mid.tile([128, 128], BF16)
        ones_sb = mid.tile([128, 128], BF16)
        v_tok = s2.tile([128, 32, 128], BF16)    # v token-major per 128-slice
        yT = s2.tile([128, QPK, TOK], BF16)      # attention out, feature-major
        wq_sb = s1.tile([128, 6, 32, 128], BF16)

        nc.sync.dma_start(ident_sb[:], ident[:])
        nc.sync.dma_start(tri_sb[:], tri[:])
        nc.sync.dma_start(ones_sb[:], ones[:])
        # prewarm the exp table on ACT while DMAs stream
        warm = s2.tile([128, 128], BF16, tag="warm")
        nc.scalar.activation(warm[:], tri_sb[:], AF.Exp, scale=1.0)

        state = {"x": {}, "cs": {}, "vtmp": {}, "wp": None}

        def dma_chunk_piece(c, qi):
            xq = s1.tile([128, 4, 512], BF16, tag="x", bufs=15,
                         name=f"x{c}_{qi}")
            nc.sync.dma_start(xq[:], xt[:, c, qi * 4:(qi + 1) * 4, :])
            state["x"].setdefault(c, {})[qi] = xq

        def dma_chunk_cs(c):
            cst = s1.tile([128, 2, 512], BF16, tag="cs", bufs=1, name=f"cs{c}")
            nc.sync.dma_start(cst[:], cs[:, c % 4, :, :])
            state["cs"][c] = cst

        def dma_chunk(c):
            """Issue DMAs for chunk c's x (8 eighth tiles) + cos/sin."""
            for qi in range(8):
                dma_chunk_piece(c, qi)
            dma_chunk_cs(c)

        # startup: interleave wq m0/m1 pieces with x chunk-0 pieces so the
        # first k-loop can follow the DMA arrival curve
        for qr in range(4):
            nc.sync.dma_start(wq_sb[:, 0, qr * 8:(qr + 1) * 8, :],
                              wq[:, 0, qr * 8:(qr + 1) * 8, :])
            dma_chunk_piece(0, qr * 2)
            dma_chunk_piece(0, qr * 2 + 1)
            nc.sync.dma_start(wq_sb[:, 1, qr * 8:(qr + 1) * 8, :],
                              wq[:, 1, qr * 8:(qr + 1) * 8, :])
        dma_chunk_cs(0)
        for m in range(2, 6):
            for half in range(2):
                nc.sync.dma_start(wq_sb[:, m, half * 16:(half + 1) * 16, :],
                                  wq[:, m, half * 16:(half + 1) * 16, :])
        dma_chunk(1)

        def emit_s1_mm(c, m, kk, ps):
            xq = state["x"][c][kk // 4]
            nc.tensor.matmul(ps[:], wq_sb[:, m, kk, :], xq[:, kk % 4, :],
                             start=(kk == 0), stop=(kk == 31))

        def emit_s1_epilogue(c, m, ps):
            t0 = c * 512
            cst = state["cs"][c]
            if m == 5:                       # v: stage for transpose
                vtmp = s1.tile([128, 512], BF16, tag="vtmp", bufs=2,
                               name=f"vt{c}")
                nc.vector.tensor_copy(vtmp[:], ps[:])
                state["vtmp"][c] = vtmp
            else:                            # q heads 0-3 / k: rope
                t1 = s1.tile([128, 512], BF16, tag="t1", bufs=2, name="t1")
                nc.vector.tensor_mul(t1[:], ps[:], cst[:, 0, :])
                u = s1.tile([128, 512], BF16, tag="u", bufs=2, name="u")
                nc.vector.tensor_mul(u[0:64, :], ps[64:128, :],
                                     cst[0:64, 1, :])
                nc.vector.tensor_mul(u[64:128, :], ps[0:64, :],
                                     cst[64:128, 1, :])
                if m < 4:
                    dst = qT[:, m, t0:t0 + 512]
                else:
                    dst = kT[:, t0:t0 + 512]
                nc.vector.tensor_add(dst, t1[:], u[:])

        def emit_transpose_unit(c):
            """Transpose chunk c's v into token-major v_tok, scratching in the
            4-bank scores-quad psum tile (bitcast to bf16)."""
            vtmp = state["vtmp"][c]
            psq = pscr.tile([128, 4, 512], F32, tag="s", name=f"tp{c}")
            pb = psq.bitcast(BF16)           # [128, 4, 1024]
            for s in range(4):
                dst = pb[:, s, 0:128]
                nc.tensor.transpose(dst, vtmp[:, s * 128:(s + 1) * 128],
                                    ident_sb[:])
                nc.vector.tensor_copy(v_tok[:, c * 4 + s, :], dst)

        # fill queue: list of (kind, cycles, emit_fn)
        fillq = []

        def push_s1_chunk(c):
            if c + 1 < NCH:
                fillq.append(("dma", 0, lambda cc=c + 1: dma_chunk(cc)))
            for m in range(6):
                holder = {}
                for gi, (k0, k1) in enumerate(((0, 12), (12, 24), (24, 32))):
                    def fn(cc=c, mm=m, kk0=k0, kk1=k1, gi=gi, h=holder):
                        if gi == 0:
                            h["ps"] = pf.tile([128, 512], F32, tag="f",
                                              name=f"s1p{cc}_{mm}")
                        for kk in range(kk0, kk1):
                            emit_s1_mm(cc, mm, kk, h["ps"])
                        if kk1 == 32:
                            emit_s1_epilogue(cc, mm, h["ps"])
                    fillq.append(("s1", (k1 - k0) * 512, fn))
            fillq.append(("s1", 4 * 430, lambda cc=c: emit_transpose_unit(cc)))

        def push_proj(b, tcq):
            for ti in range(b * 16 + tcq * 4, b * 16 + tcq * 4 + 4):
                for ccg in range(8):
                    def fn(t=ti, cg=ccg):
                        wp_sb = state["wp"]
                        ps_p = pf.tile([128, 512], F32, tag="f",
                                       name=f"pj{t}_{cg}")
                        for h in range(QPK):
                            nc.tensor.matmul(
                                ps_p[:], yT[:, h, t * 128:(t + 1) * 128],
                                wp_sb[:, h, cg * 512:(cg + 1) * 512],
                                start=(h == 0), stop=(h == 3))
                        ob = state["s3"].tile([128, 512], BF16, tag="ob",
                                              bufs=4, name=f"ob{t}_{cg}")
                        if (t * 8 + cg) % 2 == 0:
                            nc.vector.tensor_copy(ob[:], ps_p[:])
                        else:
                            nc.scalar.activation(ob[:], ps_p[:], AF.Copy)
                        nc.sync.dma_start(
                            out[t * 128:(t + 1) * 128,
                                cg * 512:(cg + 1) * 512], ob[:])
                    fillq.append(("pj", 4 * 512, fn))

        def emit_fill(budget, kinds):
            spent = 0
            while fillq and spent < budget and fillq[0][0] in kinds:
                kind, cyc, fn = fillq.pop(0)
                fn()
                spent += cyc
            return spent

        # ---------------- attention group ----------------
        def attention_group(b, tcq, h, fill_budget, kinds):
            t0g = b * T + tcq * 512
            n_s = (tcq + 1) * 4
            ps_o = po.tile([128, 512], F32, tag="o", name=f"o{b}{tcq}{h}")
            ps_d = pd.tile([128, 512], F32, tag="d", name=f"d{b}{tcq}{h}")
            for q in range(tcq + 1):
                diag = (q == tcq)
                ps_s = pscr.tile([128, 4, 512], F32, tag="s",
                                 name=f"s{b}{tcq}{h}{q}")
                offs = []
                for j in range(4):
                    si = q * 4 + j
                    off = 128 * j if diag else 0
                    offs.append(off)
                    s0g = b * T + si * 128
                    nc.tensor.matmul(
                        ps_s[:, j, off:512], kT[:, s0g:s0g + 128],
                        qT[:, h, t0g + off:t0g + 512],
                        start=True, stop=True)
                pt = s2.tile([128, 4, 512], BF16, tag="pt", bufs=2,
                             name=f"pt{b}{tcq}{h}{q}")
                nc.scalar.activation(pt[:], ps_s[:], AF.Exp, scale=SCALE)
                emit_fill(fill_budget, kinds)
                if diag:
                    for j in range(4):
                        o = 128 * j
                        nc.gpsimd.tensor_mul(pt[:, j, o:o + 128],
                                             pt[:, j, o:o + 128], tri_sb[:])
                for j in range(4):
                    si = q * 4 + j
                    off = offs[j]
                    nc.tensor.matmul(
                        ps_o[:, off:512], v_tok[:, b * 16 + si, :],
                        pt[:, j, off:512],
                        start=(si == 0), stop=(si == n_s - 1))
                    nc.tensor.matmul(
                        ps_d[:, off:512], ones_sb[:], pt[:, j, off:512],
                        start=(si == 0), stop=(si == n_s - 1))
            rden = s2.tile([128, 512], F32, tag="rd", bufs=1, name="rden")
            nc.vector.reciprocal_approx_fast(rden[:], ps_d[:])
            nc.vector.tensor_mul(yT[:, h, t0g:t0g + 512], ps_o[:], rden[:])

        # ================= emission =================
        # seg1: chunks 0-3 dense (b=0 qkv).  m-tiles processed in pairs with
        # split k-halves so late-arriving x quarters get 2x the DMA lead.
        for c in range(4):
            for ma, mb in ((0, 1), (2, 3), (4, 5)):
                if ma == 0 and c + 2 < 4:
                    dma_chunk(c + 2)
                psa = pf.tile([128, 512], F32, tag="f", name=f"c{c}m{ma}")
                psb = pf.tile([128, 512], F32, tag="f", name=f"c{c}m{mb}")
                for kk in range(16):
                    emit_s1_mm(c, ma, kk, psa)
                for kk in range(16):
                    emit_s1_mm(c, mb, kk, psb)
                for kk in range(16, 32):
                    emit_s1_mm(c, ma, kk, psa)
                emit_s1_epilogue(c, ma, psa)
                for kk in range(16, 32):
                    emit_s1_mm(c, mb, kk, psb)
                emit_s1_epilogue(c, mb, psb)
                if ma == 2 and c >= 1:
                    emit_transpose_unit(c - 1)
        emit_transpose_unit(3)

        # queue b=1 qkv as fill for seg2; prefetch chunk 4 now
        dma_chunk(4)
        for c in range(4, 8):
            push_s1_chunk(c)

        s1_cycles = sum(cyc for _, cyc, _ in fillq)
        n_quads = QPK * sum(tcq + 1 for tcq in range(4))   # 40
        budget0 = s1_cycles // n_quads + 1

        # seg2: b=0 attention + b=1 qkv fills
        for tcq in range(4):
            for h in range(QPK):
                attention_group(0, tcq, h, budget0, ("dma", "s1"))
            push_proj(0, tcq)

        # make sure all b=1 qkv work is emitted before b=1 attention
        while fillq and fillq[0][0] in ("dma", "s1"):
            _, _, fn = fillq.pop(0)
            fn()

        # close stage-1 pool, open proj pool; wproj arrives during b=1 attn
        s1_cm.__exit__(None, None, None)
        s3_cm = tc.tile_pool(name="s3", bufs=1)
        s3 = s3_cm.__enter__()
        state["s3"] = s3
        wp_sb = s3.tile([128, QPK, C], BF16)
        state["wp"] = wp_sb
        for h in range(QPK):
            for half in range(2):
                nc.sync.dma_start(wp_sb[:, h, half * 2048:(half + 1) * 2048],
                                  wp[:, h, half * 2048:(half + 1) * 2048])

        pj_cycles = sum(cyc for _, cyc, _ in fillq)
        budget1 = pj_cycles // n_quads + 1

        # seg3: b=1 attention + b=0 proj fills
        for tcq in range(4):
            for h in range(QPK):
                attention_group(1, tcq, h, budget1, ("dma", "s1", "pj"))
            push_proj(1, tcq)

        # seg4: drain remaining proj
        while fillq:
            _, _, fn = fillq.pop(0)
            fn()

        for cm in (s3_cm, pd_cm, po_cm, pscr_cm, pf_cm, s2_cm, mid_cm):
            cm.__exit__(None, None, None)
    nc.compile()
    return nc


def _prep_inputs(x, cos, sin, Wqkv, Wproj):
    bf = ml_dtypes.bfloat16
    # x: [B,T,C] -> xT [C, TOK] -> [128p, chunk, 32ko, 512]
    xTn = x.reshape(TOK, C).T.astype(bf)                  # [C, TOK]
    xpack = np.ascontiguousarray(
        xTn.reshape(32, 128, NCH, 512).transpose(1, 2, 0, 3))
    # cos/sin: [T, 128] -> feature-major slices [128, 4tcq, 2, 512]
    cosT = cos.T.astype(np.float32)                       # [128, T]
    sinT = sin.T.astype(np.float32)
    sinb = np.concatenate([-sinT[0:64], sinT[64:128]], axis=0)
    cspack = np.empty((128, 4, 2, 512), dtype=np.float32)
    for tc in range(4):
        cspack[:, tc, 0, :] = cosT[:, tc * 512:(tc + 1) * 512]
        cspack[:, tc, 1, :] = sinb[:, tc * 512:(tc + 1) * 512]
    cspack = cspack.astype(bf)
    p = np.arange(128)[:, None]
    f = np.arange(128)[None, :]
    tri = (p <= f).astype(bf)                             # tril mask (kv<=q)
    ones = np.ones([128, 128], dtype=bf)
    ident = np.eye(128, dtype=np.float32).astype(bf)
    in_maps = []
    for g in range(N_CORES):
        Wg = np.ascontiguousarray(Wqkv[:, g * GCOLS:(g + 1) * GCOLS])
        # [C, 768] -> [128p, 6m, 32ko, 128]
        wqp = np.ascontiguousarray(
            Wg.reshape(32, 128, 6, 128).transpose(1, 2, 0, 3).astype(bf))
        Wpg = Wproj[g * 512:(g + 1) * 512, :]             # [512, C]
        wpp = np.ascontiguousarray(
            Wpg.reshape(QPK, 128, C).transpose(1, 0, 2).astype(bf))
        in_maps.append({
            "xt": xpack, "wq": wqp, "wp": wpp, "cs": cspack,
            "tri": tri, "ones": ones, "ident": ident,
        })
    return in_maps


def kernel(x, cos, sin, Wqkv, Wproj, _trace=False):
    global _NC_CACHE
    x = np.asarray(x, dtype=np.float32)
    cos = np.asarray(cos, dtype=np.float32)
    sin = np.asarray(sin, dtype=np.float32)
    Wqkv = np.asarray(Wqkv, dtype=np.float32)
    Wproj = np.asarray(Wproj, dtype=np.float32)
    if _NC_CACHE is None:
        _NC_CACHE = build_nc()
    nc = _NC_CACHE
    in_maps = _prep_inputs(x, cos, sin, Wqkv, Wproj)
    res = run_bass_kernel_spmd(nc, in_maps, core_ids=list(range(N_CORES)),
                               trace=_trace)
    acc = np.zeros([TOK, C], dtype=np.float32)
    for r in res.results:
        acc += r["out"].astype(np.float32)
    if _trace:
        kernel._last_exec_ns = res.exec_time_ns
        kernel._last_trace = res.instructions_and_trace
    return acc.reshape(B, T, C)
